# revision 64
# baseline (speedup 1.0000x reference)
"""Trainium2 Bass kernel for nn_Attention_9981503996487.

Single-layer attention prefill (B=1, S=4096, H=2048, 16 q-heads, 4 kv-heads,
D=128, RoPE, causal, GQA, empty KV cache at cache_position=0).

Sharding (tensor parallel over heads): core c owns q-heads {2c, 2c+1} and
kv-head c//2.  wq/wk/wv split column-wise, wo row-wise; each core computes a
partial o_proj output over its 256 head-channels and the host sums the 8
partials (the "all-reduce").

v2 design (all bf16 matmul datapath, fp32 PSUM):
  The kernel is a software-pipelined loop over 8 query chunks of 512.  Per
  super-step ci it emits, interleaved at sub-chunk granularity so the PE
  never idles behind the ACT-bound softmax:
    B(ci)   flash-style causal attention for the chunk (PE scores + PV,
            ACT exp singles, DVE bf16 denominator quad-adds + masks),
    C(ci-1) o_proj for the previous chunk (PE matmuls, Pool-engine PSUM
            evacuation, batched DRAM stores),
    A(ci+1) QKV projection + RoPE for the next chunk (PE matmuls, Pool
            evacuation, DVE RoPE, V transposed to [s,d] via DMA XBAR).
  Denominators: p tiles are exp'd to bf16 SBUF singles; full k-tiles are
  accumulated elementwise into a [128, 4, 512] bf16 quad accumulator on the
  DVE (2x bf16 mode); an all-ones bf16 matmul reduces/broadcasts the quad
  into fp32 PSUM, followed by a fast reciprocal and one normalize multiply.
"""

import math

import numpy as np

S = 4096
HID = 2048
D = 128
NCORES = 8
CH = 512          # query / s-chunk width
NCH = S // CH     # 8 chunks
NT = HID // 128   # 16 contraction tiles
SCALE = 1.0 / math.sqrt(D)


def _build_nc():
    import concourse.bacc as bacc
    import concourse.mybir as mybir
    import concourse.tile as tile

    f32 = mybir.dt.float32
    DT = mybir.dt.bfloat16
    EXP = mybir.ActivationFunctionType.Exp

    nc = bacc.Bacc("TRN2", target_bir_lowering=False, debug=False)

    hT = nc.dram_tensor("hT", [HID, S], DT, kind="ExternalInput")
    wcat = nc.dram_tensor("wcat", [HID, 512], DT, kind="ExternalInput")
    wo2 = nc.dram_tensor("wo2", [256, HID], DT, kind="ExternalInput")
    cosT = nc.dram_tensor("cosT", [128, S], DT, kind="ExternalInput")
    sinTs = nc.dram_tensor("sinTs", [128, S], DT, kind="ExternalInput")
    tri = nc.dram_tensor("tri", [128, 128], DT, kind="ExternalInput")
    mneg = nc.dram_tensor("mneg", [128, 128], DT, kind="ExternalInput")
    ones = nc.dram_tensor("ones", [128, 128], DT, kind="ExternalInput")
    out = nc.dram_tensor("out", [S, HID], DT, kind="ExternalOutput")

    with tile.TileContext(nc) as tc:
        with (
            tc.tile_pool(name="persist", bufs=1) as persist,
            tc.tile_pool(name="ld", bufs=2) as ld,
            tc.tile_pool(name="rope", bufs=2) as rope,
            tc.tile_pool(name="pq", bufs=20) as pqp,
            tc.tile_pool(name="dn", bufs=2) as dnp,
            tc.tile_pool(name="osb", bufs=4) as osbp,
            tc.tile_pool(name="ob", bufs=3) as obp,
            tc.tile_pool(name="psA", bufs=2, space="PSUM") as psA,
            tc.tile_pool(name="psS", bufs=2, space="PSUM") as psS,
            tc.tile_pool(name="psT", bufs=2, space="PSUM") as psT,
            tc.tile_pool(name="psO", bufs=2, space="PSUM") as psO,
        ):
            qt0 = persist.tile([128, S], DT, name="qt0")
            qt1 = persist.tile([128, S], DT, name="qt1")
            ktt = persist.tile([128, S], DT, name="ktt")
            vsb = persist.tile([128, S], DT, name="vsb")
            tri_sb = persist.tile([128, 128], DT, name="tri_sb")
            ones_sb = persist.tile([128, 128], DT, name="ones_sb")
            wcat_sbs = []
            for g in range(4):
                wct = persist.tile([128, NT // 4, 512], DT,
                                   name=f"wcat_sb{g}", uniquify=False)
                wcat_sbs.append(wct)
            wo_sb0 = persist.tile([128, HID], DT, name="wo_sb0")
            wo_sb1 = persist.tile([128, HID], DT, name="wo_sb1")
            # chunk-0's first contraction group loads first so the PE can
            # start as early as possible after the cold DMA ramp
            for t in range(4):
                nc.sync.dma_start(wcat_sbs[0][:, t, :],
                                  wcat[t * 128:(t + 1) * 128, :])
            def load_consts():
                nc.sync.dma_start(tri_sb[:], tri[:])
                nc.sync.dma_start(ones_sb[:], ones[:])
                nc.sync.dma_start(wo_sb0[:], wo2[0:128, :])
                nc.sync.dma_start(wo_sb1[:], wo2[128:256, :])

            qdest = [qt0, qt1, ktt]
            ld_state = {}
            ot_tiles = {}

            # ---------------- Stage A: loads / QKV / RoPE / V ----------------
            def a_load(ci, interleave_wcat=False):
                s0 = ci * CH
                hts = []
                for g in range(4):
                    ht = ld.tile([128, NT // 4, CH], DT, name=f"htile{g}",
                                 tag=f"ht{g}")
                    hts.append(ht)
                    if interleave_wcat and g == 0:
                        # cold start: small separate DMAs land sooner
                        for t in range(4):
                            nc.sync.dma_start(
                                ht[:, t, :],
                                hT[t * 128:(t + 1) * 128, s0:s0 + CH],
                            )
                        # remaining preamble loads ride behind the urgent ones
                        for t in range(4, NT):
                            nc.sync.dma_start(
                                wcat_sbs[t // 4][:, t % 4, :],
                                wcat[t * 128:(t + 1) * 128, :])
                    else:
                        nc.sync.dma_start(
                            ht[:, :, :],
                            hT[4 * g * 128:(4 * g + 4) * 128,
                               s0:s0 + CH].rearrange("(t p) c -> p t c", t=4),
                        )
                cos_c = ld.tile([128, CH], DT, name="cos_c", tag="cc")
                nc.sync.dma_start(cos_c[:], cosT[:, s0:s0 + CH])
                sin_c = ld.tile([128, CH], DT, name="sin_c", tag="sc")
                nc.sync.dma_start(sin_c[:], sinTs[:, s0:s0 + CH])
                ld_state[ci] = (hts, cos_c, sin_c)

            def a_subunits(ci, o):
                """QKV projection output o as a list of ~1us sub-closures."""
                s0 = ci * CH
                st = {}

                def mm(t_lo):
                    def run():
                        if t_lo == 0:
                            st["ps"] = psA.tile([128, CH], f32, name="psa",
                                                tag="a")
                        hts, cos_c, sin_c = ld_state[ci]
                        for t in range(t_lo, t_lo + 4):
                            nc.tensor.matmul(
                                st["ps"][:],
                                wcat_sbs[t // 4][:, t % 4,
                                                 o * 128:(o + 1) * 128],
                                hts[t // 4][:, t % 4, :],
                                start=(t == 0), stop=(t == NT - 1),
                            )
                    return run

                def evac():
                    ps = st["ps"]
                    if o < 3:
                        x_sb = rope.tile([128, CH], DT, name="x_sb", tag="x")
                        nc.vector.tensor_copy(x_sb[:], ps[:])
                        swap = rope.tile([128, CH], DT, name="swap", tag="sw")
                        nc.gpsimd.dma_start(swap[0:64, :], x_sb[64:128, :])
                        nc.gpsimd.dma_start(swap[64:128, :], x_sb[0:64, :])
                        st["x"] = x_sb
                        st["sw"] = swap
                    else:
                        xv = rope.tile([128, CH], DT, name="xv", tag="xv")
                        nc.vector.tensor_copy(xv[:], ps[:])
                        st["x"] = xv

                def fold():
                    hts, cos_c, sin_c = ld_state[ci]
                    if o < 3:
                        t1 = rope.tile([128, CH], DT, name="t1", tag="t1")
                        nc.vector.tensor_mul(t1[:], st["x"][:], cos_c[:])
                        t2 = rope.tile([128, CH], DT, name="t2", tag="t2")
                        nc.vector.tensor_mul(t2[:], st["sw"][:], sin_c[:])
                        nc.vector.tensor_add(qdest[o][:, s0:s0 + CH],
                                             t1[:], t2[:])
                    else:
                        for b in range(4):
                            kt = 4 * ci + b
                            nc.sync.dma_start(
                                vsb[:, kt * 128:(kt + 1) * 128],
                                st["x"][:, b * 128:(b + 1) * 128],
                                transpose=True,
                            )

                def mm_last():
                    mm(12)()
                    evac()

                return [mm(0), mm(4), mm(8), mm_last, fold]

            # ---------------- Stage B: attention for chunk ci ----------------
            def head_steps(ci, h):
                """Skewed emit/consume step closures for one head."""
                s0 = ci * CH
                n_kt = 4 * (ci + 1)
                qt = [qt0, qt1][h]
                st = {}

                def emit(kt):
                    ridx = kt - 4 * ci
                    off = max(ridx, 0) * 128
                    stp = psS.tile([128, CH], f32, name="stp", tag="s")
                    nc.tensor.matmul(
                        stp[:, off:CH],
                        ktt[:, kt * 128:(kt + 1) * 128],
                        qt[:, s0 + off:s0 + CH],
                        start=True, stop=True,
                    )
                    p = pqp.tile([128, CH], DT, name="p_sb", tag="pq")
                    st[kt] = p
                    nc.scalar.activation(
                        p[:, off:CH], stp[:, off:CH], EXP, scale=SCALE
                    )
                    if ridx >= 0:
                        # causal mask on the diagonal 128-strip: post-exp
                        # multiplicative on the otherwise-idle Pool engine;
                        # the 2-step skew hides the latency before PV reads p
                        nc.gpsimd.tensor_mul(
                            p[:, off:off + 128], p[:, off:off + 128],
                            tri_sb[:],
                        )

                def consume(kt):
                    ridx = kt - 4 * ci
                    off = max(ridx, 0) * 128
                    p = st.pop(kt)
                    nc.tensor.matmul(
                        st["ot"][:, off:CH],
                        vsb[:, kt * 128:(kt + 1) * 128],
                        p[:, off:CH],
                        start=(kt == 0), stop=(kt == n_kt - 1),
                    )
                    # denominator: elementwise bf16 accumulate (DVE 2x mode)
                    if kt == 0:
                        nc.vector.tensor_copy(st["dn"][:], p[:])
                    else:
                        nc.vector.tensor_add(
                            st["dn"][:, off:CH], st["dn"][:, off:CH],
                            p[:, off:CH],
                        )

                def head_start():
                    st["ot"] = psT.tile([128, CH], f32, name="otp", tag="ot")
                    st["dn"] = dnp.tile([128, CH], DT, name="dn", tag="dn")

                def fin_bc():
                    bc = psO.tile([128, CH], f32, name="bc", tag="ops")
                    nc.tensor.matmul(bc[:], ones_sb[:], st["dn"][:],
                                     start=True, stop=True)
                    recip = dnp.tile([128, CH], f32, name="recip", tag="rc")
                    nc.vector.reciprocal_approx_fast(recip[:], bc[:])
                    st["rc"] = recip

                def fin_norm():
                    ot_sb = osbp.tile([128, CH], DT, name="ot_sb", tag="os")
                    nc.vector.tensor_mul(ot_sb[:], st["ot"][:], st["rc"][:])
                    ot_tiles[(ci, h)] = ot_sb

                def first_step():
                    head_start()
                    emit(0)
                    if n_kt > 1:
                        emit(1)

                def mk(kt_e, kt_c):
                    def run():
                        if kt_e is not None:
                            emit(kt_e)
                        if kt_c is not None:
                            consume(kt_c)
                    return run

                def last_step():
                    consume(n_kt - 1)

                # 2-step skew: PV(kt) runs two emissions after exp(kt)
                steps = [first_step]
                for i in range(2, n_kt):
                    steps.append(mk(i, i - 2))
                steps.append(mk(None, n_kt - 2) if n_kt > 1 else None)
                steps.append(last_step)
                return [s for s in steps if s], (fin_bc, fin_norm)

            # ---------------- Stage C: o_proj for chunk ci ----------------
            def c_subunits(ci, st_i):
                """o_proj row-tile st_i as a list of ~1us sub-closures."""
                row = (ci * 4 + st_i) * 128

                def hc_step(hc):
                    def run():
                        ots = [ot_tiles[(ci, 0)], ot_tiles[(ci, 1)]]
                        if hc == 0:
                            ot_tiles[("osb", ci, st_i)] = obp.tile(
                                [128, HID], DT, name="o_sb", tag="ob")
                        o_sb = ot_tiles[("osb", ci, st_i)]
                        ops = psO.tile([128, 512], f32, name="ops", tag="ops")
                        nc.tensor.matmul(
                            ops[:], ots[0][:, st_i * 128:(st_i + 1) * 128],
                            wo_sb0[:, hc * 512:(hc + 1) * 512],
                            start=True, stop=False,
                        )
                        nc.tensor.matmul(
                            ops[:], ots[1][:, st_i * 128:(st_i + 1) * 128],
                            wo_sb1[:, hc * 512:(hc + 1) * 512],
                            start=False, stop=True,
                        )
                        if hc == 1:
                            nc.scalar.copy(o_sb[:, hc * 512:(hc + 1) * 512],
                                           ops[:])
                        else:
                            nc.vector.tensor_copy(
                                o_sb[:, hc * 512:(hc + 1) * 512], ops[:])
                        if hc == 3:
                            eng = nc.sync if ci >= 6 else nc.gpsimd
                            eng.dma_start(out[row:row + 128, :], o_sb[:])
                    return run

                return [hc_step(hc) for hc in range(4)]

            # ---------------- pipelined emission ----------------
            def ratio_merge(core, fill):
                """Spread fill closures evenly among core closures."""
                seq = []
                ratio = len(fill) / max(len(core), 1)
                acc = 0.0
                fi = 0
                for item in core:
                    seq.append(item)
                    acc += ratio
                    while fi < len(fill) and acc >= 1.0:
                        seq.append(fill[fi])
                        fi += 1
                        acc -= 1.0
                seq.extend(fill[fi:])
                return seq

            a_load(0, interleave_wcat=True)
            load_consts()
            a_load(1)
            for o in range(4):
                for sub in a_subunits(0, o):
                    sub()
            c_queue = []
            fin_prev = None      # head-1 finale of the previous chunk
            for ci in range(NCH):
                fill = []
                if ci + 2 < NCH:
                    fill.append(lambda ci=ci: a_load(ci + 2))
                if ci + 1 < NCH:
                    for o in range(4):
                        fill.extend(a_subunits(ci + 1, o))
                if ci - 1 >= 0:
                    for st_i in range(4):
                        c_queue.extend(c_subunits(ci - 1, st_i))
                # hold back some o_proj work during super-steps 5/6 so the
                # ACT-bound final super-step still has PE work to chew on
                npop = len(c_queue) if ci not in (5, 6) else 8
                cpops = c_queue[:npop]
                del c_queue[:npop]

                steps0, fin0 = head_steps(ci, 0)
                steps1, fin1 = head_steps(ci, 1)
                # weave the o_proj sub-units evenly among the QKV sub-units
                # (but only after the previous head-1 finale has run, since
                # o_proj consumes its normalized outputs)
                woven = fill[:8] + ratio_merge(fill[8:], cpops)
                full_fill = list(woven)
                if fin_prev is not None:
                    full_fill[1:1] = [fin_prev[0]]
                    full_fill[3:3] = [fin_prev[1]]
                half = (len(full_fill) * len(steps0)) // \
                    max(len(steps0) + len(steps1), 1)
                seq = ratio_merge(steps0, full_fill[:half])
                # head-0 bc+recip land ~80% into segment 1 (den long since
                # accumulated) so the recip isn't hot at the next super-step
                seg1_fill = full_fill[half:]
                cut = (len(seg1_fill) * 4) // 5
                seg1_fill = seg1_fill[:cut] + [fin0[0]] + seg1_fill[cut:]
                seq += ratio_merge(steps1, seg1_fill) + [fin0[1]]
                for item in seq:
                    item()
                fin_prev = fin1
            fin_prev[0]()
            fin_prev[1]()
            for item in c_queue:
                item()
            for st_i in range(4):
                for sub in c_subunits(NCH - 1, st_i):
                    sub()

    nc.finalize()
    return nc


def _host_prep(hidden_states, cos, sin, position_ids, wq, wk, wv, wo):
    """Build the 8 per-core input maps."""
    import ml_dtypes
    np_dt = ml_dtypes.bfloat16

    hidden = np.asarray(hidden_states, dtype=np.float32)[0]        # [S, HID]
    hT = np.ascontiguousarray(hidden.T).astype(np_dt)              # [HID, S]
    pos = np.asarray(position_ids)[0].astype(np.int64)             # [S]
    cos_np = np.asarray(cos, dtype=np.float32)[pos]                # [S, 64]
    sin_np = np.asarray(sin, dtype=np.float32)[pos]
    cos_full = np.concatenate([cos_np, cos_np], axis=1)            # [S, 128]
    sin_full = np.concatenate([sin_np, sin_np], axis=1)
    cosT = np.ascontiguousarray(cos_full.T).astype(np_dt)          # [128, S]
    sinTs = np.ascontiguousarray(sin_full.T)
    sinTs[0:64, :] *= -1.0                                         # sign fold
    sinTs = sinTs.astype(np_dt)

    # multiplicative causal mask for the diagonal 128-strips: keep k <= j
    kk = np.arange(128)[:, None]
    jj = np.arange(128)[None, :]
    tri = (kk <= jj).astype(np_dt)                                 # [128, 128]
    mneg = np.where(kk > jj, -1e8, 0.0).astype(np_dt)              # unused
    ones = np.ones((128, 128), dtype=np_dt)

    wq_np = np.asarray(wq, dtype=np.float32)
    wk_np = np.asarray(wk, dtype=np.float32)
    wv_np = np.asarray(wv, dtype=np.float32)
    wo_np = np.asarray(wo, dtype=np.float32)

    in_maps = []
    for c in range(NCORES):
        h0 = 2 * c
        g = c // 2
        wcat = np.ascontiguousarray(np.concatenate(
            [
                wq_np[:, h0 * D:(h0 + 1) * D],
                wq_np[:, (h0 + 1) * D:(h0 + 2) * D],
                wk_np[:, g * D:(g + 1) * D],
                wv_np[:, g * D:(g + 1) * D],
            ],
            axis=1,
        )).astype(np_dt)                                           # [HID, 512]
        wo2 = np.ascontiguousarray(
            wo_np[h0 * D:(h0 + 2) * D, :]
        ).astype(np_dt)                                            # [256, HID]
        in_maps.append({
            "hT": hT,
            "wcat": wcat,
            "wo2": wo2,
            "cosT": cosT,
            "sinTs": sinTs,
            "tri": tri,
            "mneg": mneg,
            "ones": ones,
        })
    return in_maps


_NC_CACHE = [None]


def _run(inputs, trace=False, tmpdir=None):
    from concourse import bass_utils

    in_maps = _host_prep(
        inputs["hidden_states"], inputs["cos"], inputs["sin"],
        inputs["position_ids"], inputs["wq"], inputs["wk"], inputs["wv"],
        inputs["wo"],
    )
    if _NC_CACHE[0] is None:
        _NC_CACHE[0] = _build_nc()
    nc = _NC_CACHE[0]
    res = bass_utils.run_bass_kernel_spmd(
        nc, in_maps, core_ids=list(range(NCORES)), trace=trace, tmpdir=tmpdir,
    )
    acc = res.results[0]["out"].astype(np.float32)
    for c in range(1, NCORES):
        acc = acc + res.results[c]["out"].astype(np.float32)
    return acc.reshape(1, S, HID), res


def kernel(**inputs):
    out, _ = _run(inputs, trace=False)
    return out


# revision 70
# speedup vs baseline: 1.0294x; 1.0294x over previous
"""Trainium2 Bass kernel for nn_Attention_9981503996487.

Single-layer attention prefill (B=1, S=4096, H=2048, 16 q-heads, 4 kv-heads,
D=128, RoPE, causal, GQA, empty KV cache at cache_position=0).

Sharding (tensor parallel over heads): core c owns q-heads {2c, 2c+1} and
kv-head c//2.  wq/wk/wv split column-wise, wo row-wise; each core computes a
partial o_proj output over its 256 head-channels and the host sums the 8
partials (the "all-reduce").

v2 design (all bf16 matmul datapath, fp32 PSUM):
  The kernel is a software-pipelined loop over 8 query chunks of 512.  Per
  super-step ci it emits, interleaved at sub-chunk granularity so the PE
  never idles behind the ACT-bound softmax:
    B(ci)   flash-style causal attention for the chunk (PE scores + PV,
            ACT exp singles, DVE bf16 denominator quad-adds + masks),
    C(ci-1) o_proj for the previous chunk (PE matmuls, Pool-engine PSUM
            evacuation, batched DRAM stores),
    A(ci+1) QKV projection + RoPE for the next chunk (PE matmuls, Pool
            evacuation, DVE RoPE, V transposed to [s,d] via DMA XBAR).
  Denominators: p tiles are exp'd to bf16 SBUF singles; full k-tiles are
  accumulated elementwise into a [128, 4, 512] bf16 quad accumulator on the
  DVE (2x bf16 mode); an all-ones bf16 matmul reduces/broadcasts the quad
  into fp32 PSUM, followed by a fast reciprocal and one normalize multiply.
"""

import math

import numpy as np

S = 4096
HID = 2048
D = 128
NCORES = 8
CH = 512          # query / s-chunk width
NCH = S // CH     # 8 chunks
NT = HID // 128   # 16 contraction tiles
SCALE = 1.0 / math.sqrt(D)


def _build_nc():
    import concourse.bacc as bacc
    import concourse.mybir as mybir
    import concourse.tile as tile

    f32 = mybir.dt.float32
    DT = mybir.dt.bfloat16
    EXP = mybir.ActivationFunctionType.Exp

    nc = bacc.Bacc("TRN2", target_bir_lowering=False, debug=False)

    hT = nc.dram_tensor("hT", [HID, S], DT, kind="ExternalInput")
    wcat = nc.dram_tensor("wcat", [HID, 512], DT, kind="ExternalInput")
    wo2 = nc.dram_tensor("wo2", [256, HID], DT, kind="ExternalInput")
    cosT = nc.dram_tensor("cosT", [128, S], DT, kind="ExternalInput")
    sinTs = nc.dram_tensor("sinTs", [128, S], DT, kind="ExternalInput")
    tri = nc.dram_tensor("tri", [128, 128], DT, kind="ExternalInput")
    mneg = nc.dram_tensor("mneg", [128, 128], DT, kind="ExternalInput")
    ones = nc.dram_tensor("ones", [128, 128], DT, kind="ExternalInput")
    out = nc.dram_tensor("out", [S, HID], DT, kind="ExternalOutput")

    with tile.TileContext(nc) as tc:
        with (
            tc.tile_pool(name="persist", bufs=1) as persist,
            tc.tile_pool(name="ld", bufs=2) as ld,
            tc.tile_pool(name="rope", bufs=2) as rope,
            tc.tile_pool(name="pq", bufs=20) as pqp,
            tc.tile_pool(name="dn", bufs=2) as dnp,
            tc.tile_pool(name="osb", bufs=6) as osbp,
            tc.tile_pool(name="ob", bufs=3) as obp,
            tc.tile_pool(name="psA", bufs=2, space="PSUM") as psA,
            tc.tile_pool(name="psS", bufs=2, space="PSUM") as psS,
            tc.tile_pool(name="psT", bufs=2, space="PSUM") as psT,
            tc.tile_pool(name="psO", bufs=2, space="PSUM") as psO,
        ):
            qt0 = persist.tile([128, S], DT, name="qt0")
            qt1 = persist.tile([128, S], DT, name="qt1")
            ktt = persist.tile([128, S], DT, name="ktt")
            vsb = persist.tile([128, S], DT, name="vsb")
            id_sb = persist.tile([128, 128], DT, name="id_sb")
            mneg_sb = persist.tile([128, 128], DT, name="mneg_sb")
            ones_sb = persist.tile([128, 128], DT, name="ones_sb")
            wcat_sbs = []
            for g in range(4):
                wct = persist.tile([128, NT // 4, 512], DT,
                                   name=f"wcat_sb{g}", uniquify=False)
                wcat_sbs.append(wct)
            wo_sb0 = persist.tile([128, HID], DT, name="wo_sb0")
            wo_sb1 = persist.tile([128, HID], DT, name="wo_sb1")
            pre_ht0 = ld.tile([128, NT // 4, CH], DT, name="htile0",
                              tag="ht0")
            # chunk-0's first contraction group loads first, weights and
            # activations interleaved, so the PE can start as early as
            # possible after the cold DMA ramp
            for t in range(4):
                nc.sync.dma_start(wcat_sbs[0][:, t, :],
                                  wcat[t * 128:(t + 1) * 128, :])
                nc.sync.dma_start(pre_ht0[:, t, :],
                                  hT[t * 128:(t + 1) * 128, 0:CH])
            def load_consts():
                nc.sync.dma_start(id_sb[:], tri[:])
                nc.sync.dma_start(mneg_sb[:], mneg[:])
                nc.sync.dma_start(ones_sb[:], ones[:])
                nc.sync.dma_start(wo_sb0[:], wo2[0:128, :])
                nc.sync.dma_start(wo_sb1[:], wo2[128:256, :])

            qdest = [qt0, qt1, ktt]
            ld_state = {}
            ot_tiles = {}

            # ---------------- Stage A: loads / QKV / RoPE / V ----------------
            def a_load(ci, interleave_wcat=False):
                s0 = ci * CH
                hts = []
                for g in range(4):
                    if interleave_wcat and g == 0:
                        # group 0 was preloaded before the remaining weights
                        hts.append(pre_ht0)
                        for t in range(4, NT):
                            nc.sync.dma_start(
                                wcat_sbs[t // 4][:, t % 4, :],
                                wcat[t * 128:(t + 1) * 128, :])
                        continue
                    ht = ld.tile([128, NT // 4, CH], DT, name=f"htile{g}",
                                 tag=f"ht{g}")
                    hts.append(ht)
                    if False:
                        pass
                    else:
                        nc.sync.dma_start(
                            ht[:, :, :],
                            hT[4 * g * 128:(4 * g + 4) * 128,
                               s0:s0 + CH].rearrange("(t p) c -> p t c", t=4),
                        )
                cos_c = ld.tile([128, CH], DT, name="cos_c", tag="cc")
                nc.sync.dma_start(cos_c[:], cosT[:, s0:s0 + CH])
                sin_c = ld.tile([128, CH], DT, name="sin_c", tag="sc")
                nc.sync.dma_start(sin_c[:], sinTs[:, s0:s0 + CH])
                ld_state[ci] = (hts, cos_c, sin_c)

            def a_subunits(ci, o):
                """QKV projection output o as a list of ~1us sub-closures."""
                s0 = ci * CH
                st = {}

                def mm(t_lo):
                    def run():
                        if t_lo == 0:
                            st["ps"] = psA.tile([128, CH], f32, name="psa",
                                                tag="a")
                        hts, cos_c, sin_c = ld_state[ci]
                        for t in range(t_lo, t_lo + 4):
                            nc.tensor.matmul(
                                st["ps"][:],
                                wcat_sbs[t // 4][:, t % 4,
                                                 o * 128:(o + 1) * 128],
                                hts[t // 4][:, t % 4, :],
                                start=(t == 0), stop=(t == NT - 1),
                            )
                    return run

                def evac():
                    ps = st["ps"]
                    if o < 3:
                        x_sb = rope.tile([128, CH], DT, name="x_sb", tag="x")
                        nc.vector.tensor_copy(x_sb[:], ps[:])
                        swap = rope.tile([128, CH], DT, name="swap", tag="sw")
                        nc.gpsimd.dma_start(swap[0:64, :], x_sb[64:128, :])
                        nc.gpsimd.dma_start(swap[64:128, :], x_sb[0:64, :])
                        st["x"] = x_sb
                        st["sw"] = swap
                    else:
                        xv = rope.tile([128, CH], DT, name="xv", tag="xv")
                        nc.vector.tensor_copy(xv[:], ps[:])
                        st["x"] = xv

                def fold():
                    hts, cos_c, sin_c = ld_state[ci]
                    if o < 3:
                        t1 = rope.tile([128, CH], DT, name="t1", tag="t1")
                        nc.vector.tensor_mul(t1[:], st["x"][:], cos_c[:])
                        t2 = rope.tile([128, CH], DT, name="t2", tag="t2")
                        nc.vector.tensor_mul(t2[:], st["sw"][:], sin_c[:])
                        nc.vector.tensor_add(qdest[o][:, s0:s0 + CH],
                                             t1[:], t2[:])
                    else:
                        for b in range(4):
                            kt = 4 * ci + b
                            nc.sync.dma_start(
                                vsb[:, kt * 128:(kt + 1) * 128],
                                st["x"][:, b * 128:(b + 1) * 128],
                                transpose=True,
                            )

                def mm_last():
                    mm(12)()
                    evac()

                return [mm(0), mm(4), mm(8), mm_last, fold]

            # ---------------- Stage B: attention for chunk ci ----------------
            def head_steps(ci, h):
                """Skewed emit/consume step closures for one head."""
                s0 = ci * CH
                n_kt = 4 * (ci + 1)
                qt = [qt0, qt1][h]
                st = {}

                def emit(kt):
                    ridx = kt - 4 * ci
                    off = max(ridx, 0) * 128
                    stp = psS.tile([128, CH], f32, name="stp", tag="s")
                    nc.tensor.matmul(
                        stp[:, off:CH],
                        ktt[:, kt * 128:(kt + 1) * 128],
                        qt[:, s0 + off:s0 + CH],
                        start=True, stop=(ridx < 0),
                    )
                    if ridx >= 0:
                        # accumulate the -1e8 strict-upper mask into the
                        # causal 128-strip (PE-side masking, pre-exp)
                        nc.tensor.matmul(
                            stp[:, off:off + 128], id_sb[:], mneg_sb[:],
                            start=False, stop=True,
                        )
                    p = pqp.tile([128, CH], DT, name="p_sb", tag="pq")
                    st[kt] = p
                    nc.scalar.activation(
                        p[:, off:CH], stp[:, off:CH], EXP, scale=SCALE
                    )

                def consume(kt):
                    ridx = kt - 4 * ci
                    off = max(ridx, 0) * 128
                    p = st.pop(kt)
                    nc.tensor.matmul(
                        st["ot"][:, off:CH],
                        vsb[:, kt * 128:(kt + 1) * 128],
                        p[:, off:CH],
                        start=(kt == 0), stop=(kt == n_kt - 1),
                    )
                    # denominator: elementwise bf16 accumulate (DVE 2x mode)
                    if kt == 0:
                        nc.vector.tensor_copy(st["dn"][:], p[:])
                    else:
                        nc.vector.tensor_add(
                            st["dn"][:, off:CH], st["dn"][:, off:CH],
                            p[:, off:CH],
                        )

                def head_start():
                    st["ot"] = psT.tile([128, CH], f32, name="otp", tag="ot")
                    st["dn"] = dnp.tile([128, CH], DT, name="dn", tag="dn")

                def fin_bc():
                    bc = psO.tile([128, CH], f32, name="bc", tag="ops")
                    nc.tensor.matmul(bc[:], ones_sb[:], st["dn"][:],
                                     start=True, stop=True)
                    recip = dnp.tile([128, CH], f32, name="recip", tag="rc")
                    nc.vector.reciprocal_approx_fast(recip[:], bc[:])
                    st["rc"] = recip

                def fin_norm():
                    ot_sb = osbp.tile([128, CH], DT, name="ot_sb", tag="os")
                    nc.vector.tensor_mul(ot_sb[:], st["ot"][:], st["rc"][:])
                    ot_tiles[(ci, h)] = ot_sb

                def first_step():
                    head_start()
                    emit(0)
                    if n_kt > 1:
                        emit(1)

                def mk(kt_e, kt_c):
                    def run():
                        if kt_e is not None:
                            emit(kt_e)
                        if kt_c is not None:
                            consume(kt_c)
                    return run

                def last_step():
                    consume(n_kt - 1)

                # 2-step skew: PV(kt) runs two emissions after exp(kt)
                steps = [first_step]
                for i in range(2, n_kt):
                    steps.append(mk(i, i - 2))
                steps.append(mk(None, n_kt - 2) if n_kt > 1 else None)
                steps.append(last_step)
                return [s for s in steps if s], (fin_bc, fin_norm)

            # ---------------- Stage C: o_proj for chunk ci ----------------
            def c_subunits(ci, st_i):
                """o_proj row-tile st_i as a list of ~1us sub-closures."""
                row = (ci * 4 + st_i) * 128

                def hc_step(hc):
                    def run():
                        ots = [ot_tiles[(ci, 0)], ot_tiles[(ci, 1)]]
                        if hc == 0:
                            ot_tiles[("osb", ci, st_i)] = obp.tile(
                                [128, HID], DT, name="o_sb", tag="ob")
                        o_sb = ot_tiles[("osb", ci, st_i)]
                        ops = psO.tile([128, 512], f32, name="ops", tag="ops")
                        nc.tensor.matmul(
                            ops[:], ots[0][:, st_i * 128:(st_i + 1) * 128],
                            wo_sb0[:, hc * 512:(hc + 1) * 512],
                            start=True, stop=False,
                        )
                        nc.tensor.matmul(
                            ops[:], ots[1][:, st_i * 128:(st_i + 1) * 128],
                            wo_sb1[:, hc * 512:(hc + 1) * 512],
                            start=False, stop=True,
                        )
                        if hc == 1:
                            nc.scalar.copy(o_sb[:, hc * 512:(hc + 1) * 512],
                                           ops[:])
                        else:
                            nc.vector.tensor_copy(
                                o_sb[:, hc * 512:(hc + 1) * 512], ops[:])
                        if hc == 3:
                            eng = nc.sync if ci >= 6 else nc.gpsimd
                            eng.dma_start(out[row:row + 128, :], o_sb[:])
                    return run

                return [hc_step(hc) for hc in range(4)]

            # ---------------- pipelined emission ----------------
            def ratio_merge(core, fill):
                """Spread fill closures evenly among core closures."""
                seq = []
                ratio = len(fill) / max(len(core), 1)
                acc = 0.0
                fi = 0
                for item in core:
                    seq.append(item)
                    acc += ratio
                    while fi < len(fill) and acc >= 1.0:
                        seq.append(fill[fi])
                        fi += 1
                        acc -= 1.0
                seq.extend(fill[fi:])
                return seq

            a_load(0, interleave_wcat=True)
            load_consts()
            a_load(1)
            for o in range(4):
                for sub in a_subunits(0, o):
                    sub()
            c_queue = []
            fin_prev = None      # head-1 finale of the previous chunk
            for ci in range(NCH):
                fill = []
                if ci + 2 < NCH:
                    fill.append(lambda ci=ci: a_load(ci + 2))
                if ci + 1 < NCH:
                    for o in range(4):
                        fill.extend(a_subunits(ci + 1, o))
                if ci - 1 >= 0:
                    for st_i in range(4):
                        c_queue.extend(c_subunits(ci - 1, st_i))
                # hold back some o_proj work during super-steps 5/6 so the
                # ACT-bound final super-step still has PE work to chew on
                npop = len(c_queue) if ci not in (4, 5, 6) else 8
                cpops = c_queue[:npop]
                del c_queue[:npop]

                steps0, fin0 = head_steps(ci, 0)
                steps1, fin1 = head_steps(ci, 1)
                # weave the o_proj sub-units evenly among the QKV sub-units
                # (but only after the previous head-1 finale has run, since
                # o_proj consumes its normalized outputs)
                woven = fill[:4] + ratio_merge(fill[4:], cpops)
                full_fill = list(woven)
                if fin_prev is not None:
                    full_fill[1:1] = [fin_prev[0]]
                    full_fill[3:3] = [fin_prev[1]]
                half = (len(full_fill) * len(steps0)) // \
                    max(len(steps0) + len(steps1), 1)
                seq = ratio_merge(steps0, full_fill[:half])
                # head-0 bc+recip land ~80% into segment 1 (den long since
                # accumulated) so the recip isn't hot at the next super-step
                seg1_fill = full_fill[half:]
                cut = (len(seg1_fill) * 4) // 5
                seg1_fill = seg1_fill[:cut] + [fin0[0]] + seg1_fill[cut:]
                seq += ratio_merge(steps1, seg1_fill) + [fin0[1]]
                for item in seq:
                    item()
                fin_prev = fin1
            fin_prev[0]()
            fin_prev[1]()
            for item in c_queue:
                item()
            for st_i in range(4):
                for sub in c_subunits(NCH - 1, st_i):
                    sub()

    nc.finalize()
    return nc


def _host_prep(hidden_states, cos, sin, position_ids, wq, wk, wv, wo):
    """Build the 8 per-core input maps."""
    import ml_dtypes
    np_dt = ml_dtypes.bfloat16

    hidden = np.asarray(hidden_states, dtype=np.float32)[0]        # [S, HID]
    hT = np.ascontiguousarray(hidden.T).astype(np_dt)              # [HID, S]
    pos = np.asarray(position_ids)[0].astype(np.int64)             # [S]
    cos_np = np.asarray(cos, dtype=np.float32)[pos]                # [S, 64]
    sin_np = np.asarray(sin, dtype=np.float32)[pos]
    cos_full = np.concatenate([cos_np, cos_np], axis=1)            # [S, 128]
    sin_full = np.concatenate([sin_np, sin_np], axis=1)
    cosT = np.ascontiguousarray(cos_full.T).astype(np_dt)          # [128, S]
    sinTs = np.ascontiguousarray(sin_full.T)
    sinTs[0:64, :] *= -1.0                                         # sign fold
    sinTs = sinTs.astype(np_dt)

    # identity (for the PE-side mask accumulate) and the additive causal
    # mask for the diagonal 128-strips: -1e8 where k > j (invisible)
    kk = np.arange(128)[:, None]
    jj = np.arange(128)[None, :]
    tri = np.eye(128).astype(np_dt)                                # identity
    mneg = np.where(kk > jj, -1e8, 0.0).astype(np_dt)              # [128, 128]
    ones = np.ones((128, 128), dtype=np_dt)

    wq_np = np.asarray(wq, dtype=np.float32)
    wk_np = np.asarray(wk, dtype=np.float32)
    wv_np = np.asarray(wv, dtype=np.float32)
    wo_np = np.asarray(wo, dtype=np.float32)

    in_maps = []
    for c in range(NCORES):
        h0 = 2 * c
        g = c // 2
        wcat = np.ascontiguousarray(np.concatenate(
            [
                wq_np[:, h0 * D:(h0 + 1) * D],
                wq_np[:, (h0 + 1) * D:(h0 + 2) * D],
                wk_np[:, g * D:(g + 1) * D],
                wv_np[:, g * D:(g + 1) * D],
            ],
            axis=1,
        )).astype(np_dt)                                           # [HID, 512]
        wo2 = np.ascontiguousarray(
            wo_np[h0 * D:(h0 + 2) * D, :]
        ).astype(np_dt)                                            # [256, HID]
        in_maps.append({
            "hT": hT,
            "wcat": wcat,
            "wo2": wo2,
            "cosT": cosT,
            "sinTs": sinTs,
            "tri": tri,
            "mneg": mneg,
            "ones": ones,
        })
    return in_maps


_NC_CACHE = [None]


def _run(inputs, trace=False, tmpdir=None):
    from concourse import bass_utils

    in_maps = _host_prep(
        inputs["hidden_states"], inputs["cos"], inputs["sin"],
        inputs["position_ids"], inputs["wq"], inputs["wk"], inputs["wv"],
        inputs["wo"],
    )
    if _NC_CACHE[0] is None:
        _NC_CACHE[0] = _build_nc()
    nc = _NC_CACHE[0]
    res = bass_utils.run_bass_kernel_spmd(
        nc, in_maps, core_ids=list(range(NCORES)), trace=trace, tmpdir=tmpdir,
    )
    acc = res.results[0]["out"].astype(np.float32)
    for c in range(1, NCORES):
        acc = acc + res.results[c]["out"].astype(np.float32)
    return acc.reshape(1, S, HID), res


def kernel(**inputs):
    out, _ = _run(inputs, trace=False)
    return out


# revision 72
# speedup vs baseline: 1.0355x; 1.0059x over previous
"""Trainium2 Bass kernel for nn_Attention_9981503996487.

Single-layer attention prefill (B=1, S=4096, H=2048, 16 q-heads, 4 kv-heads,
D=128, RoPE, causal, GQA, empty KV cache at cache_position=0).

Sharding (tensor parallel over heads): core c owns q-heads {2c, 2c+1} and
kv-head c//2.  wq/wk/wv split column-wise, wo row-wise; each core computes a
partial o_proj output over its 256 head-channels and the host sums the 8
partials (the "all-reduce").

v2 design (all bf16 matmul datapath, fp32 PSUM):
  The kernel is a software-pipelined loop over 8 query chunks of 512.  Per
  super-step ci it emits, interleaved at sub-chunk granularity so the PE
  never idles behind the ACT-bound softmax:
    B(ci)   flash-style causal attention for the chunk (PE scores + PV,
            ACT exp singles, DVE bf16 denominator quad-adds + masks),
    C(ci-1) o_proj for the previous chunk (PE matmuls, Pool-engine PSUM
            evacuation, batched DRAM stores),
    A(ci+1) QKV projection + RoPE for the next chunk (PE matmuls, Pool
            evacuation, DVE RoPE, V transposed to [s,d] via DMA XBAR).
  Denominators: p tiles are exp'd to bf16 SBUF singles; full k-tiles are
  accumulated elementwise into a [128, 4, 512] bf16 quad accumulator on the
  DVE (2x bf16 mode); an all-ones bf16 matmul reduces/broadcasts the quad
  into fp32 PSUM, followed by a fast reciprocal and one normalize multiply.
"""

import math

import numpy as np

S = 4096
HID = 2048
D = 128
NCORES = 8
CH = 512          # query / s-chunk width
NCH = S // CH     # 8 chunks
NT = HID // 128   # 16 contraction tiles
SCALE = 1.0 / math.sqrt(D)


def _build_nc():
    import concourse.bacc as bacc
    import concourse.mybir as mybir
    import concourse.tile as tile

    f32 = mybir.dt.float32
    DT = mybir.dt.bfloat16
    EXP = mybir.ActivationFunctionType.Exp

    nc = bacc.Bacc("TRN2", target_bir_lowering=False, debug=False)

    hT = nc.dram_tensor("hT", [HID, S], DT, kind="ExternalInput")
    wcat = nc.dram_tensor("wcat", [HID, 512], DT, kind="ExternalInput")
    wo2 = nc.dram_tensor("wo2", [256, HID], DT, kind="ExternalInput")
    cosT = nc.dram_tensor("cosT", [128, S], DT, kind="ExternalInput")
    sinTs = nc.dram_tensor("sinTs", [128, S], DT, kind="ExternalInput")
    tri = nc.dram_tensor("tri", [128, 128], DT, kind="ExternalInput")
    mneg = nc.dram_tensor("mneg", [128, 128], DT, kind="ExternalInput")
    ones = nc.dram_tensor("ones", [128, 128], DT, kind="ExternalInput")
    out = nc.dram_tensor("out", [S, HID], DT, kind="ExternalOutput")

    with tile.TileContext(nc) as tc:
        with (
            tc.tile_pool(name="persist", bufs=1) as persist,
            tc.tile_pool(name="ld", bufs=2) as ld,
            tc.tile_pool(name="rope", bufs=2) as rope,
            tc.tile_pool(name="pq", bufs=20) as pqp,
            tc.tile_pool(name="dn", bufs=2) as dnp,
            tc.tile_pool(name="osb", bufs=6) as osbp,
            tc.tile_pool(name="ob", bufs=3) as obp,
            tc.tile_pool(name="psA", bufs=2, space="PSUM") as psA,
            tc.tile_pool(name="psS", bufs=2, space="PSUM") as psS,
            tc.tile_pool(name="psT", bufs=2, space="PSUM") as psT,
            tc.tile_pool(name="psO", bufs=2, space="PSUM") as psO,
        ):
            qt0 = persist.tile([128, S], DT, name="qt0")
            qt1 = persist.tile([128, S], DT, name="qt1")
            ktt = persist.tile([128, S], DT, name="ktt")
            vsb = persist.tile([128, S], DT, name="vsb")
            id_sb = persist.tile([128, 128], DT, name="id_sb")
            mneg_sb = persist.tile([128, 128], DT, name="mneg_sb")
            ones_sb = persist.tile([128, 128], DT, name="ones_sb")
            wcat_sbs = []
            for g in range(4):
                wct = persist.tile([128, NT // 4, 512], DT,
                                   name=f"wcat_sb{g}", uniquify=False)
                wcat_sbs.append(wct)
            wo_sb0 = persist.tile([128, HID], DT, name="wo_sb0")
            wo_sb1 = persist.tile([128, HID], DT, name="wo_sb1")
            pre_ht0 = ld.tile([128, NT // 4, CH], DT, name="htile0",
                              tag="ht0")
            # chunk-0's first contraction group loads first, weights and
            # activations interleaved, so the PE can start as early as
            # possible after the cold DMA ramp
            for t in range(4):
                nc.sync.dma_start(wcat_sbs[0][:, t, :],
                                  wcat[t * 128:(t + 1) * 128, :])
                nc.sync.dma_start(pre_ht0[:, t, :],
                                  hT[t * 128:(t + 1) * 128, 0:CH])
            def load_consts():
                nc.sync.dma_start(id_sb[:], tri[:])
                nc.sync.dma_start(mneg_sb[:], mneg[:])
                nc.sync.dma_start(ones_sb[:], ones[:])
                nc.sync.dma_start(wo_sb0[:], wo2[0:128, :])
                nc.sync.dma_start(wo_sb1[:], wo2[128:256, :])

            qdest = [qt0, qt1, ktt]
            ld_state = {}
            ot_tiles = {}

            # ---------------- Stage A: loads / QKV / RoPE / V ----------------
            def a_load(ci, interleave_wcat=False):
                s0 = ci * CH
                hts = []
                for g in range(4):
                    if interleave_wcat and g == 0:
                        # group 0 was preloaded before the remaining weights
                        hts.append(pre_ht0)
                        for t in range(4, NT):
                            nc.sync.dma_start(
                                wcat_sbs[t // 4][:, t % 4, :],
                                wcat[t * 128:(t + 1) * 128, :])
                        continue
                    ht = ld.tile([128, NT // 4, CH], DT, name=f"htile{g}",
                                 tag=f"ht{g}")
                    hts.append(ht)
                    if False:
                        pass
                    else:
                        nc.sync.dma_start(
                            ht[:, :, :],
                            hT[4 * g * 128:(4 * g + 4) * 128,
                               s0:s0 + CH].rearrange("(t p) c -> p t c", t=4),
                        )
                cos_c = ld.tile([128, CH], DT, name="cos_c", tag="cc")
                nc.sync.dma_start(cos_c[:], cosT[:, s0:s0 + CH])
                sin_c = ld.tile([128, CH], DT, name="sin_c", tag="sc")
                nc.sync.dma_start(sin_c[:], sinTs[:, s0:s0 + CH])
                ld_state[ci] = (hts, cos_c, sin_c)

            def a_subunits(ci, o):
                """QKV projection output o as a list of ~1us sub-closures."""
                s0 = ci * CH
                st = {}

                def mm(t_lo):
                    def run():
                        if t_lo == 0:
                            st["ps"] = psA.tile([128, CH], f32, name="psa",
                                                tag="a")
                        hts, cos_c, sin_c = ld_state[ci]
                        for t in range(t_lo, t_lo + 4):
                            nc.tensor.matmul(
                                st["ps"][:],
                                wcat_sbs[t // 4][:, t % 4,
                                                 o * 128:(o + 1) * 128],
                                hts[t // 4][:, t % 4, :],
                                start=(t == 0), stop=(t == NT - 1),
                            )
                    return run

                def evac():
                    ps = st["ps"]
                    if o < 3:
                        x_sb = rope.tile([128, CH], DT, name="x_sb", tag="x")
                        nc.vector.tensor_copy(x_sb[:], ps[:])
                        swap = rope.tile([128, CH], DT, name="swap", tag="sw")
                        nc.gpsimd.dma_start(swap[0:64, :], x_sb[64:128, :])
                        nc.gpsimd.dma_start(swap[64:128, :], x_sb[0:64, :])
                        st["x"] = x_sb
                        st["sw"] = swap
                    else:
                        xv = rope.tile([128, CH], DT, name="xv", tag="xv")
                        nc.vector.tensor_copy(xv[:], ps[:])
                        st["x"] = xv

                def fold():
                    hts, cos_c, sin_c = ld_state[ci]
                    if o < 3:
                        t1 = rope.tile([128, CH], DT, name="t1", tag="t1")
                        nc.vector.tensor_mul(t1[:], st["x"][:], cos_c[:])
                        t2 = rope.tile([128, CH], DT, name="t2", tag="t2")
                        nc.vector.tensor_mul(t2[:], st["sw"][:], sin_c[:])
                        nc.vector.tensor_add(qdest[o][:, s0:s0 + CH],
                                             t1[:], t2[:])
                    else:
                        for b in range(4):
                            kt = 4 * ci + b
                            nc.sync.dma_start(
                                vsb[:, kt * 128:(kt + 1) * 128],
                                st["x"][:, b * 128:(b + 1) * 128],
                                transpose=True,
                            )

                def mm_last():
                    mm(12)()
                    evac()

                return [mm(0), mm(4), mm(8), mm_last, fold]

            # ---------------- Stage B: attention for chunk ci ----------------
            def head_steps(ci, h):
                """Skewed emit/consume step closures for one head."""
                s0 = ci * CH
                n_kt = 4 * (ci + 1)
                qt = [qt0, qt1][h]
                st = {}

                def emit(kt):
                    ridx = kt - 4 * ci
                    off = max(ridx, 0) * 128
                    stp = psS.tile([128, CH], f32, name="stp", tag="s")
                    nc.tensor.matmul(
                        stp[:, off:CH],
                        ktt[:, kt * 128:(kt + 1) * 128],
                        qt[:, s0 + off:s0 + CH],
                        start=True, stop=True,
                    )
                    p = pqp.tile([128, CH], DT, name="p_sb", tag="pq")
                    st[kt] = p
                    nc.scalar.activation(
                        p[:, off:CH], stp[:, off:CH], EXP, scale=SCALE
                    )
                    if ridx >= 0:
                        # causal mask on the diagonal 128-strip: post-exp
                        # multiplicative on the DVE; the 2-step skew hides
                        # the latency before PV/den read p
                        nc.vector.tensor_mul(
                            p[:, off:off + 128], p[:, off:off + 128],
                            id_sb[:],
                        )

                def consume(kt):
                    ridx = kt - 4 * ci
                    off = max(ridx, 0) * 128
                    p = st.pop(kt)
                    nc.tensor.matmul(
                        st["ot"][:, off:CH],
                        vsb[:, kt * 128:(kt + 1) * 128],
                        p[:, off:CH],
                        start=(kt == 0), stop=(kt == n_kt - 1),
                    )
                    # denominator: elementwise bf16 accumulate (DVE 2x mode)
                    if kt == 0:
                        nc.vector.tensor_copy(st["dn"][:], p[:])
                    else:
                        nc.vector.tensor_add(
                            st["dn"][:, off:CH], st["dn"][:, off:CH],
                            p[:, off:CH],
                        )

                def head_start():
                    st["ot"] = psT.tile([128, CH], f32, name="otp", tag="ot")
                    st["dn"] = dnp.tile([128, CH], DT, name="dn", tag="dn")

                def fin_bc():
                    bc = psO.tile([128, CH], f32, name="bc", tag="ops")
                    nc.tensor.matmul(bc[:], ones_sb[:], st["dn"][:],
                                     start=True, stop=True)
                    recip = dnp.tile([128, CH], f32, name="recip", tag="rc")
                    nc.vector.reciprocal_approx_fast(recip[:], bc[:])
                    st["rc"] = recip

                def fin_norm():
                    ot_sb = osbp.tile([128, CH], DT, name="ot_sb", tag="os")
                    nc.vector.tensor_mul(ot_sb[:], st["ot"][:], st["rc"][:])
                    ot_tiles[(ci, h)] = ot_sb

                def first_step():
                    head_start()
                    emit(0)
                    if n_kt > 1:
                        emit(1)

                def mk(kt_e, kt_c):
                    def run():
                        if kt_e is not None:
                            emit(kt_e)
                        if kt_c is not None:
                            consume(kt_c)
                    return run

                def last_step():
                    consume(n_kt - 1)

                # 2-step skew: PV(kt) runs two emissions after exp(kt)
                steps = [first_step]
                for i in range(2, n_kt):
                    steps.append(mk(i, i - 2))
                steps.append(mk(None, n_kt - 2) if n_kt > 1 else None)
                steps.append(last_step)
                return [s for s in steps if s], (fin_bc, fin_norm)

            # ---------------- Stage C: o_proj for chunk ci ----------------
            def c_subunits(ci, st_i):
                """o_proj row-tile st_i as a list of ~1us sub-closures."""
                row = (ci * 4 + st_i) * 128

                def hc_step(hc):
                    def run():
                        ots = [ot_tiles[(ci, 0)], ot_tiles[(ci, 1)]]
                        if hc == 0:
                            ot_tiles[("osb", ci, st_i)] = obp.tile(
                                [128, HID], DT, name="o_sb", tag="ob")
                        o_sb = ot_tiles[("osb", ci, st_i)]
                        ops = psO.tile([128, 512], f32, name="ops", tag="ops")
                        nc.tensor.matmul(
                            ops[:], ots[0][:, st_i * 128:(st_i + 1) * 128],
                            wo_sb0[:, hc * 512:(hc + 1) * 512],
                            start=True, stop=False,
                        )
                        nc.tensor.matmul(
                            ops[:], ots[1][:, st_i * 128:(st_i + 1) * 128],
                            wo_sb1[:, hc * 512:(hc + 1) * 512],
                            start=False, stop=True,
                        )
                        if hc == 1:
                            nc.scalar.copy(o_sb[:, hc * 512:(hc + 1) * 512],
                                           ops[:])
                        else:
                            nc.vector.tensor_copy(
                                o_sb[:, hc * 512:(hc + 1) * 512], ops[:])
                        if hc == 3:
                            eng = nc.sync if ci >= 6 else nc.gpsimd
                            eng.dma_start(out[row:row + 128, :], o_sb[:])
                    return run

                return [hc_step(hc) for hc in range(4)]

            # ---------------- pipelined emission ----------------
            def ratio_merge(core, fill):
                """Spread fill closures evenly among core closures."""
                seq = []
                ratio = len(fill) / max(len(core), 1)
                acc = 0.0
                fi = 0
                for item in core:
                    seq.append(item)
                    acc += ratio
                    while fi < len(fill) and acc >= 1.0:
                        seq.append(fill[fi])
                        fi += 1
                        acc -= 1.0
                seq.extend(fill[fi:])
                return seq

            a_load(0, interleave_wcat=True)
            load_consts()
            a_load(1)
            for o in range(4):
                for sub in a_subunits(0, o):
                    sub()
            c_queue = []
            fin_prev = None      # head-1 finale of the previous chunk
            for ci in range(NCH):
                fill = []
                if ci + 2 < NCH:
                    fill.append(lambda ci=ci: a_load(ci + 2))
                if ci + 1 < NCH:
                    for o in range(4):
                        fill.extend(a_subunits(ci + 1, o))
                if ci - 1 >= 0:
                    for st_i in range(4):
                        c_queue.extend(c_subunits(ci - 1, st_i))
                # hold back some o_proj work during super-steps 5/6 so the
                # ACT-bound final super-step still has PE work to chew on
                npop = len(c_queue) if ci not in (4, 5, 6) else 8
                cpops = c_queue[:npop]
                del c_queue[:npop]

                steps0, fin0 = head_steps(ci, 0)
                steps1, fin1 = head_steps(ci, 1)
                # weave the o_proj sub-units evenly among the QKV sub-units
                # (but only after the previous head-1 finale has run, since
                # o_proj consumes its normalized outputs)
                woven = fill[:4] + ratio_merge(fill[4:], cpops)
                full_fill = list(woven)
                if fin_prev is not None:
                    full_fill[1:1] = [fin_prev[0]]
                    full_fill[3:3] = [fin_prev[1]]
                half = (len(full_fill) * len(steps0)) // \
                    max(len(steps0) + len(steps1), 1)
                seq = ratio_merge(steps0, full_fill[:half])
                # head-0 bc+recip land ~80% into segment 1 (den long since
                # accumulated) so the recip isn't hot at the next super-step
                seg1_fill = full_fill[half:]
                cut = (len(seg1_fill) * 4) // 5
                seg1_fill = seg1_fill[:cut] + [fin0[0]] + seg1_fill[cut:]
                seq += ratio_merge(steps1, seg1_fill) + [fin0[1]]
                for item in seq:
                    item()
                fin_prev = fin1
            fin_prev[0]()
            fin_prev[1]()
            for item in c_queue:
                item()
            for st_i in range(4):
                for sub in c_subunits(NCH - 1, st_i):
                    sub()

    nc.finalize()
    return nc


def _host_prep(hidden_states, cos, sin, position_ids, wq, wk, wv, wo):
    """Build the 8 per-core input maps."""
    import ml_dtypes
    np_dt = ml_dtypes.bfloat16

    hidden = np.asarray(hidden_states, dtype=np.float32)[0]        # [S, HID]
    hT = np.ascontiguousarray(hidden.T).astype(np_dt)              # [HID, S]
    pos = np.asarray(position_ids)[0].astype(np.int64)             # [S]
    cos_np = np.asarray(cos, dtype=np.float32)[pos]                # [S, 64]
    sin_np = np.asarray(sin, dtype=np.float32)[pos]
    cos_full = np.concatenate([cos_np, cos_np], axis=1)            # [S, 128]
    sin_full = np.concatenate([sin_np, sin_np], axis=1)
    cosT = np.ascontiguousarray(cos_full.T).astype(np_dt)          # [128, S]
    sinTs = np.ascontiguousarray(sin_full.T)
    sinTs[0:64, :] *= -1.0                                         # sign fold
    sinTs = sinTs.astype(np_dt)

    # identity (for the PE-side mask accumulate) and the additive causal
    # mask for the diagonal 128-strips: -1e8 where k > j (invisible)
    kk = np.arange(128)[:, None]
    jj = np.arange(128)[None, :]
    tri = (kk <= jj).astype(np_dt)                                 # keep k<=j
    mneg = np.where(kk > jj, -1e8, 0.0).astype(np_dt)              # [128, 128]
    ones = np.ones((128, 128), dtype=np_dt)

    wq_np = np.asarray(wq, dtype=np.float32)
    wk_np = np.asarray(wk, dtype=np.float32)
    wv_np = np.asarray(wv, dtype=np.float32)
    wo_np = np.asarray(wo, dtype=np.float32)

    in_maps = []
    for c in range(NCORES):
        h0 = 2 * c
        g = c // 2
        wcat = np.ascontiguousarray(np.concatenate(
            [
                wq_np[:, h0 * D:(h0 + 1) * D],
                wq_np[:, (h0 + 1) * D:(h0 + 2) * D],
                wk_np[:, g * D:(g + 1) * D],
                wv_np[:, g * D:(g + 1) * D],
            ],
            axis=1,
        )).astype(np_dt)                                           # [HID, 512]
        wo2 = np.ascontiguousarray(
            wo_np[h0 * D:(h0 + 2) * D, :]
        ).astype(np_dt)                                            # [256, HID]
        in_maps.append({
            "hT": hT,
            "wcat": wcat,
            "wo2": wo2,
            "cosT": cosT,
            "sinTs": sinTs,
            "tri": tri,
            "mneg": mneg,
            "ones": ones,
        })
    return in_maps


_NC_CACHE = [None]


def _run(inputs, trace=False, tmpdir=None):
    from concourse import bass_utils

    in_maps = _host_prep(
        inputs["hidden_states"], inputs["cos"], inputs["sin"],
        inputs["position_ids"], inputs["wq"], inputs["wk"], inputs["wv"],
        inputs["wo"],
    )
    if _NC_CACHE[0] is None:
        _NC_CACHE[0] = _build_nc()
    nc = _NC_CACHE[0]
    res = bass_utils.run_bass_kernel_spmd(
        nc, in_maps, core_ids=list(range(NCORES)), trace=trace, tmpdir=tmpdir,
    )
    acc = res.results[0]["out"].astype(np.float32)
    for c in range(1, NCORES):
        acc = acc + res.results[c]["out"].astype(np.float32)
    return acc.reshape(1, S, HID), res


def kernel(**inputs):
    out, _ = _run(inputs, trace=False)
    return out
